# revision 7
# baseline (speedup 1.0000x reference)
"""DkNN retrieval kernel for 8 trn2 NeuronCores (self-contained).

Algorithm (matches reference.py):
  xq = x/||x|| - center;  score_j = ||X_j||^2 - 2 xq.X_j;  closest = argmin_j
  neigh = [closest, tni[closest]];  counts = bincount(labels[neigh]);
  p = (1000 - bisect_left(cali, 75-counts))/1000;  creds = onehot(argmax p)*max p

Distribution: X sharded over 8 cores on the train axis (12500 rows each,
padded to 12800 with far-away fake rows). Queries replicated. Matmuls use a
3-term bf16 split (hi*Hi + hi*Lo + lo*Hi) for ~2e-7 score accuracy (bf16
alone flips ~10 argmins; fp32r ~4.5e-5 error; fp32 native is 4x slower).

Host precomputes: row norms SS (replaces a 13MB fp32 X read + 100 Square
ops), the fused label table LTAB[j] = labels[[j, tni[j]]] (replaces the
neighbor-row gather + 75 per-slot label gathers with ONE indirect DMA), and
the conformal LUT p76[v] = (1000 - bisect_left(cali, v))/1000.

Device: X hi/lo preloaded to SBUF in 4 full-width DMAs; per (super, qtile)
6 bf16 matmuls accumulate -2*xq.X into a PSUM bank, then two custom DVE ops
read the bank directly: MINRED (body=ps+ss, accum MIN -> per-super min
value) and IDX_SCAN (reversed scan -> first argmin position). Cross-core
combine via AllToAll of (val, pos); tail (label counts + conformal
p-values) on the query-owning core.

HW quirks honored: indirect_copy gathers only from low SBUF addresses
(~<32KB absolute) -> p76 LUT tile allocated side="left"; indirect DMA
supports one offset per partition per call.
"""
import os
import numpy as np

import concourse.bass as bass
import concourse.bacc as bacc
import concourse.tile as tile
import concourse.mybir as mybir
import concourse.dve_ops as dve_ops_mod
from concourse.bass_utils import run_bass_kernel_spmd
from concourse.dve_ops import DveOp, OPS
from concourse.dve_spec import Spec, Src0, Src1, C0, MaxNeg, scan, select, eq, Idx, lower
from concourse.dve_uop import DveOpSpec, AluOp
from concourse.dve_table_gen import dve_ver_for

NB_DATA = 1024
NB_TRAIN = 100000
D = 256
NB_CALI = 1000
NCORES = 8

SHARD = 12500          # real candidates per core
SHARD_PAD = 12800      # padded (fake rows score ~+1e4, never win)
SUPER = 512            # candidate columns per PSUM super-tile (1 bank)
NSUP = 25              # 25*512 = 12800
QT = 8                 # query tiles of 128

_AluOp = mybir.AluOpType


def _register_dve(name, spec):
    if name in dve_ops_mod._SUB_OPCODE_FOR_NAME:
        for op in OPS:
            if op.name == name:
                return op
    opcode = dve_ops_mod._CUSTOM_DVE_ROW_BASE + len(OPS)
    dve_ops_mod._SUB_OPCODE_FOR_NAME[name] = opcode
    ver = dve_ver_for("TRN2")
    tmp = DveOpSpec(name=name, opcode=opcode, uops=lower(spec, ver=ver), rd1_en=True)
    op = DveOp(name, spec, subdim=False, uops_sha={ver: tmp.sha(ver)})
    OPS.append(op)
    return op


def _idx_scan_spec():
    s = Src0 + Src1
    r = scan(AluOp.MIN, s, init=C0)
    body = select(eq(s, r), Idx, MaxNeg)

    def ref(in0, in1, s0, s1, imm2):
        v = (in0.astype(np.float64) + in1.astype(np.float64)).astype(np.float32)
        rm = np.minimum(np.minimum.accumulate(v, axis=-1), np.float32(s0))
        idx = np.arange(v.shape[-1], dtype=np.float64)
        sel = np.where(v == rm, idx, -3.4e38)
        return sel.astype(np.float32)

    return Spec(body=body, accum=AluOp.MAX, reference=ref)


def _minred_spec():
    def ref(in0, in1, s0, s1, imm2):
        v = (in0.astype(np.float32) + in1.astype(np.float32))
        out = v.astype(np.float32)
        acc = np.minimum(np.min(v, axis=-1), np.float32(s0))
        return out, acc

    return Spec(body=Src0 + Src1, accum=AluOp.MIN, accum_init=C0, reference=ref)


IDX_SCAN = _register_dve("IDX_SCAN_ANT", _idx_scan_spec())
MINRED = _register_dve("MINRED_ANT", _minred_spec())
dt = mybir.dt


def build_kernel():
    PHASE = int(os.environ.get("KPHASE", "3"))
    nc = bacc.Bacc("TRN2", target_bir_lowering=False, debug=False,
                   num_devices=NCORES)

    # ---- I/O ----
    xhiT = nc.dram_tensor("xhiT", [D, SHARD_PAD], dt.bfloat16, kind="ExternalInput").ap()
    xloT = nc.dram_tensor("xloT", [D, SHARD_PAD], dt.bfloat16, kind="ExternalInput").ap()
    ss_in = nc.dram_tensor("ss_in", [1, SHARD_PAD], dt.float32, kind="ExternalInput").ap()
    xq_in = nc.dram_tensor("xq_in", [NB_DATA, D], dt.float32, kind="ExternalInput").ap()
    ltab = nc.dram_tensor("ltab", [NB_TRAIN, 75], dt.int32, kind="ExternalInput").ap()
    p76_in = nc.dram_tensor("p76_in", [1, 76], dt.float32, kind="ExternalInput").ap()
    center = nc.dram_tensor("center", [1, D], dt.float32, kind="ExternalInput").ap()
    ident = nc.dram_tensor("ident", [128, 128], dt.float32, kind="ExternalInput").ap()
    dmask = nc.dram_tensor("dmask", [128, 16], dt.float32, kind="ExternalInput").ap()
    iota10 = nc.dram_tensor("iota10", [128, 10], dt.float32, kind="ExternalInput").ap()
    coff = nc.dram_tensor("coff", [128, 1], dt.float32, kind="ExternalInput").ap()
    creds_out = nc.dram_tensor("creds", [128, 10], dt.float32, kind="ExternalOutput").ap()

    with tile.TileContext(nc) as tc:
        with tc.tile_pool(name="dram", bufs=1, space="DRAM") as dpool:
            loc_d = dpool.tile([NB_DATA, 2], dt.float32)
            glob_d = dpool.tile([NCORES, 128, 2], dt.float32)

            with tc.tile_pool(name="mp", bufs=1, side="right") as mp, \
                 tc.tile_pool(name="mp2", bufs=2, side="right") as mp2, \
                 tc.tile_pool(name="pp", bufs=1, space="PSUM") as pp:

                # ===== preload X hi/lo (4 max-width DMAs) + SS broadcast =====
                xh = [mp.tile([128, SHARD_PAD], dt.bfloat16, name=f"xh{k}") for k in range(2)]
                xl = [mp.tile([128, SHARD_PAD], dt.bfloat16, name=f"xl{k}") for k in range(2)]
                for k in range(2):
                    nc.sync.dma_start(xh[k][:], xhiT[k * 128:(k + 1) * 128, :])
                    nc.sync.dma_start(xl[k][:], xloT[k * 128:(k + 1) * 128, :])
                ssB = mp.tile([128, SHARD_PAD], dt.float32)
                nc.sync.dma_start(ssB[:], ss_in[0:1, :].to_broadcast([128, SHARD_PAD]))

                # ===== query prep =====
                cb = mp.tile([128, D], dt.float32)
                crow = mp.tile([1, D], dt.float32)
                nc.sync.dma_start(crow[:], center[:, :])
                nc.gpsimd.partition_broadcast(cb[:], crow[:])
                cb2 = mp.tile([128, D], dt.float32)
                nc.scalar.mul(out=cb2[:], in_=cb[:], mul=2.0)
                idt = mp.tile([128, 128], dt.float32)
                nc.sync.dma_start(idt[:], ident[:, :])

                xqTh = [mp.tile([128, NB_DATA], dt.bfloat16, name=f"xqTh{k}") for k in range(2)]
                xqTl = [mp.tile([128, NB_DATA], dt.bfloat16, name=f"xqTl{k}") for k in range(2)]
                for t in range(QT):
                    xt = mp2.tile([128, D], dt.float32, tag="xt", name=f"xt{t}")
                    nc.sync.dma_start(xt[:], xq_in[t * 128:(t + 1) * 128, :])
                    junk = mp2.tile([128, D], dt.float32, tag="junk", name=f"junk{t}")
                    ssq = mp2.tile([128, 1], dt.float32, tag="ssq", name=f"ssq{t}")
                    nc.scalar.activation(out=junk[:], in_=xt[:],
                                         func=mybir.ActivationFunctionType.Square,
                                         accum_out=ssq[:])
                    nrm = mp2.tile([128, 1], dt.float32, tag="nrm", name=f"nrm{t}")
                    nc.scalar.sqrt(out=nrm[:], in_=ssq[:])
                    rn = mp2.tile([128, 1], dt.float32, tag="rn", name=f"rn{t}")
                    nc.vector.reciprocal(out=rn[:], in_=nrm[:])
                    nc.vector.tensor_scalar(out=rn[:], in0=rn[:], scalar1=-2.0,
                                            scalar2=None, op0=_AluOp.mult)
                    xqp = mp2.tile([128, D], dt.float32, tag="xqp", name=f"xqp{t}")
                    nc.vector.scalar_tensor_tensor(
                        out=xqp[:], in0=xt[:], scalar=rn[:, 0:1], in1=cb2[:],
                        op0=_AluOp.mult, op1=_AluOp.add)
                    for k in range(2):
                        tpt = pp.tile([128, 4 * SUPER], dt.float32, tag="ps", bufs=2,
                                      name=f"tp{t}_{k}")
                        tp = tpt[:, 0:128]
                        nc.tensor.transpose(out=tp, in_=xqp[:, k * 128:(k + 1) * 128],
                                            identity=idt[:])
                        xqf = mp2.tile([128, 128], dt.float32, tag="xqf", name=f"xqf{t}_{k}")
                        nc.scalar.copy(out=xqf[:], in_=tp)
                        nc.vector.tensor_copy(out=xqTh[k][:, t * 128:(t + 1) * 128], in_=xqf[:])
                        nc.vector.tensor_tensor(
                            out=xqTl[k][:, t * 128:(t + 1) * 128],
                            in0=xqf[:], in1=xqTh[k][:, t * 128:(t + 1) * 128],
                            op=_AluOp.subtract)

                # ===== main loop: query-tile outer, super-groups of 4 inner =====
                # groups: 6 of 4 supers (2048 cols, 4 PSUM banks) + 1 of 1
                GRP = [(g * 4, min(4, NSUP - g * 4)) for g in range((NSUP + 3) // 4)]
                NG = len(GRP)
                VAL = mp.tile([128, QT * NG], dt.float32)
                POSG = mp.tile([128, QT * NG], dt.float32)
                terms = [(xqTh, xh), (xqTh, xl), (xqTl, xh)]

                for t in range(QT):
                    for g, (s0, ns) in enumerate(GRP):
                        w = ns * SUPER
                        c0 = s0 * SUPER
                        ps = pp.tile([128, 4 * SUPER], dt.float32, tag="ps", bufs=2,
                                     name=f"ps{t}_{g}")
                        # term-major: 4 consecutive matmuls share one lhsT
                        for nmm, (lhs, rhs) in enumerate(terms):
                            for k in range(2):
                                for j in range(ns):
                                    nc.tensor.matmul(
                                        ps[:, j * SUPER:(j + 1) * SUPER],
                                        lhs[k][:, t * 128:(t + 1) * 128],
                                        rhs[k][:, c0 + j * SUPER:c0 + (j + 1) * SUPER],
                                        start=(nmm == 0 and k == 0),
                                        stop=(nmm == 2 and k == 1))
                        col = t * NG + g
                        mrd = mp2.tile([128, 4 * SUPER], dt.bfloat16, tag="mrd",
                                       name=f"mrd{t}_{g}")
                        nc.vector._custom_dve(
                            MINRED,
                            out=mrd[:, :w],
                            in0=ps[:, :w],
                            in1=ssB[:, c0:c0 + w],
                            s0=3.4e38,
                            accum_out=VAL[:, col:col + 1])
                        scr = mp2.tile([128, 4 * SUPER], dt.uint16, tag="scr",
                                       name=f"scr{t}_{g}")
                        posr = mp2.tile([128, 1], dt.float32, tag="posr",
                                        name=f"posr{t}_{g}")
                        nc.vector._custom_dve(
                            IDX_SCAN,
                            out=scr[:, :w][:, ::-1],
                            in0=ps[:, :w][:, ::-1],
                            in1=ssB[:, c0:c0 + w][:, ::-1],
                            s0=3.4e38,
                            accum_out=posr[:])
                        # true pos = (w-1) - reversed pos; global += c0
                        nc.vector.tensor_scalar(out=POSG[:, col:col + 1],
                                                in0=posr[:], scalar1=-1.0,
                                                scalar2=float(w - 1 + c0),
                                                op0=_AluOp.mult, op1=_AluOp.add)

                # ===== cross-group combine (per query-tile) =====
                gmin = mp.tile([128, 8], dt.float32)
                vview = VAL[:].rearrange("p (q s) -> p q s", q=8)
                nc.vector.tensor_reduce(gmin[:], vview, mybir.AxisListType.X,
                                        _AluOp.min)
                eqv = mp.tile([128, QT * NG], dt.uint8)
                nc.vector.tensor_tensor(
                    out=eqv[:].rearrange("p (q s) -> p q s", q=8),
                    in0=vview,
                    in1=gmin[:].unsqueeze(2).to_broadcast([128, 8, NG]),
                    op=_AluOp.is_equal)
                big = mp.tile([128, QT * NG], dt.float32)
                nc.gpsimd.memset(big[:], 1.0e9)
                selp = mp.tile([128, QT * NG], dt.float32)
                nc.vector.select(out=selp[:], mask=eqv[:], on_true=POSG[:],
                                 on_false=big[:])
                gpos = mp.tile([128, 8], dt.float32)
                nc.vector.tensor_reduce(gpos[:],
                                        selp[:].rearrange("p (q s) -> p q s", q=8),
                                        mybir.AxisListType.X, _AluOp.min)
                cof = mp.tile([128, 1], dt.float32)
                nc.sync.dma_start(cof[:], coff[:, :])
                nc.vector.tensor_scalar(out=gpos[:], in0=gpos[:],
                                        scalar1=cof[:, 0:1], scalar2=None,
                                        op0=_AluOp.add)
                locb = mp.tile([128, 16], dt.float32)
                nc.vector.tensor_copy(out=locb[:, 0::2], in_=gmin[:])
                nc.vector.tensor_copy(out=locb[:, 1::2], in_=gpos[:])
                for t in range(QT):
                    nc.sync.dma_start(loc_d[t * 128:(t + 1) * 128, :],
                                      locb[:, t * 2:t * 2 + 2])
                if PHASE == 1:
                    nc.sync.dma_start(creds_out[:, :], locb[:, :10])

            # ===== cross-core exchange + tail =====
            with tc.tile_pool(name="lo2", bufs=1, side="left") as lo2, \
                 tc.tile_pool(name="tp2", bufs=1, side="right") as tp2:
              if PHASE >= 2:
                nc.gpsimd.collective_compute(
                    "AllToAll",
                    _AluOp.bypass,
                    replica_groups=[list(range(NCORES))],
                    ins=[loc_d.opt()],
                    outs=[glob_d.opt()],
                )
                vi = tp2.tile([128, 16], dt.float32)
                nc.sync.dma_start(vi[:], glob_d[:].rearrange("r p e -> p r e"))
                vals8 = vi[:, 0::2]
                idx8 = vi[:, 1::2]
                m8 = tp2.tile([128, 1], dt.float32)
                nc.vector.tensor_reduce(m8[:], vals8, mybir.AxisListType.X,
                                        _AluOp.min)
                eq8 = tp2.tile([128, 8], dt.uint8)
                nc.vector.tensor_scalar(out=eq8[:], in0=vals8,
                                        scalar1=m8[:, 0:1], scalar2=None,
                                        op0=_AluOp.is_equal)
                big8 = tp2.tile([128, 8], dt.float32)
                nc.gpsimd.memset(big8[:], 1.0e9)
                sel8 = tp2.tile([128, 8], dt.float32)
                nc.vector.select(out=sel8[:], mask=eq8[:], on_true=idx8,
                                 on_false=big8[:])
                closf = tp2.tile([128, 1], dt.float32)
                nc.vector.tensor_reduce(closf[:], sel8[:], mybir.AxisListType.X,
                                        _AluOp.min)

                if PHASE >= 3:
                    closi = tp2.tile([128, 1], dt.int32)
                    nc.vector.tensor_copy(out=closi[:], in_=closf[:])
                    # labels of [closest, tni[closest]]: ONE row gather
                    labi = tp2.tile([128, 75], dt.int32)
                    nc.gpsimd.indirect_dma_start(
                        out=labi[:, :], out_offset=None, in_=ltab[:, :],
                        in_offset=bass.IndirectOffsetOnAxis(ap=closi[:, 0:1], axis=0))
                    labs = tp2.tile([128, 75], dt.float32)
                    nc.vector.tensor_copy(out=labs[:], in_=labi[:])

                    counts = tp2.tile([128, 10], dt.float32)
                    junk75 = tp2.tile([128, 75], dt.float32)
                    for c in range(10):
                        nc.vector.scalar_tensor_tensor(
                            out=junk75[:], in0=labs[:], scalar=float(c),
                            in1=labs[:], op0=_AluOp.is_equal, op1=_AluOp.bypass,
                            accum_out=counts[:, c:c + 1])
                    knn = tp2.tile([128, 10], dt.float32)
                    nc.vector.tensor_scalar(out=knn[:], in0=counts[:], scalar1=-1.0,
                                            scalar2=75.0, op0=_AluOp.mult,
                                            op1=_AluOp.add)

                    # conformal LUT (host-computed): p76[v] = (1000 - #(cali<v))/1000
                    p76r = tp2.tile([1, 76], dt.float32)
                    nc.sync.dma_start(p76r[:], p76_in[:, :])
                    p76b = lo2.tile([128, 76], dt.float32)  # low SBUF for gather
                    nc.gpsimd.partition_broadcast(p76b[:], p76r[:])

                    knn16 = tp2.tile([128, 10], dt.uint16)
                    nc.vector.tensor_copy(out=knn16[:], in_=knn[:])
                    gp = tp2.tile([128, 160], dt.float32)
                    nc.gpsimd.indirect_copy(out=gp[:], data=p76b[:], idxs=knn16[:],
                                            i_know_ap_gather_is_preferred=True)
                    dmt2 = tp2.tile([128, 16], dt.float32)
                    nc.sync.dma_start(dmt2[:], dmask[:, :])
                    nc.vector.tensor_tensor(
                        out=gp[:].rearrange("p (a b) -> p a b", b=16),
                        in0=gp[:].rearrange("p (a b) -> p a b", b=16),
                        in1=dmt2[:].unsqueeze(1).to_broadcast([128, 10, 16]),
                        op=_AluOp.mult)
                    pval = tp2.tile([128, 10], dt.float32)
                    nc.vector.tensor_reduce(pval[:],
                                            gp[:].rearrange("p (a b) -> p a b", b=16),
                                            mybir.AxisListType.X, _AluOp.add)

                    m10 = tp2.tile([128, 1], dt.float32)
                    nc.vector.tensor_reduce(m10[:], pval[:], mybir.AxisListType.X,
                                            _AluOp.max)
                    eqp = tp2.tile([128, 10], dt.uint8)
                    nc.vector.tensor_scalar(out=eqp[:], in0=pval[:],
                                            scalar1=m10[:, 0:1], scalar2=None,
                                            op0=_AluOp.is_equal)
                    io10 = tp2.tile([128, 10], dt.float32)
                    nc.sync.dma_start(io10[:], iota10[:, :])
                    big10 = tp2.tile([128, 10], dt.float32)
                    nc.gpsimd.memset(big10[:], 1.0e9)
                    candp = tp2.tile([128, 10], dt.float32)
                    nc.vector.select(out=candp[:], mask=eqp[:], on_true=io10[:],
                                     on_false=big10[:])
                    pred = tp2.tile([128, 1], dt.float32)
                    nc.vector.tensor_reduce(pred[:], candp[:], mybir.AxisListType.X,
                                            _AluOp.min)
                    cmask = tp2.tile([128, 10], dt.uint8)
                    nc.vector.tensor_scalar(out=cmask[:], in0=io10[:],
                                            scalar1=pred[:, 0:1], scalar2=None,
                                            op0=_AluOp.is_equal)
                    cmf = tp2.tile([128, 10], dt.float32)
                    nc.vector.tensor_copy(out=cmf[:], in_=cmask[:])
                    credst = tp2.tile([128, 10], dt.float32)
                    nc.vector.tensor_scalar(out=credst[:], in0=cmf[:],
                                            scalar1=m10[:, 0:1], scalar2=None,
                                            op0=_AluOp.mult)
                    nc.sync.dma_start(creds_out[:, :], credst[:])
                if PHASE == 2:
                    credst = tp2.tile([128, 10], dt.float32, name="credst2")
                    nc.gpsimd.memset(credst[:], 0.0)
                    nc.vector.tensor_copy(out=credst[:, 0:1], in_=closf[:])
                    nc.vector.tensor_copy(out=credst[:, 1:2], in_=m8[:])
                    nc.sync.dma_start(creds_out[:, :], credst[:])

    nc.compile()
    return nc


_NC_CACHE = None
LAST_EXEC_NS = None


def _get_nc():
    global _NC_CACHE
    if _NC_CACHE is None:
        _NC_CACHE = build_kernel()
    return _NC_CACHE


def kernel(x, X, center, train_labels, train_neighbor_index, cali_nonconformity):
    x = np.ascontiguousarray(np.asarray(x, dtype=np.float32))
    X = np.ascontiguousarray(np.asarray(X, dtype=np.float32))
    center = np.asarray(center, dtype=np.float32)
    tni = np.ascontiguousarray(np.asarray(train_neighbor_index, dtype=np.int32))
    labels = np.asarray(train_labels, dtype=np.int32)
    cali = np.asarray(cali_nonconformity, dtype=np.int32)

    import ml_dtypes

    dmask = np.zeros((128, 16), np.float32)
    for p in range(128):
        dmask[p, p % 16] = 1.0
    iota10 = np.broadcast_to(np.arange(10, dtype=np.float32), (128, 10)).copy()
    ident = np.eye(128, dtype=np.float32)
    calif = cali.astype(np.float32)
    centr = np.ascontiguousarray(center[None, :])

    # labels of [j, tni[j]] fused into one gatherable table
    ltab = np.ascontiguousarray(
        labels[np.concatenate([np.arange(NB_TRAIN, dtype=np.int32)[:, None], tni],
                              axis=1)])
    # conformal LUT over the 76 possible nonconformity values
    pos76 = np.searchsorted(cali, np.arange(76, dtype=np.int32), side='left')
    p76 = np.ascontiguousarray(
        ((NB_CALI - pos76).astype(np.float32) / float(NB_CALI))[None, :])

    in_maps = []
    for c in range(NCORES):
        Xc = np.empty((SHARD_PAD, D), np.float32)
        Xc[:SHARD] = X[c * SHARD:(c + 1) * SHARD]
        Xc[SHARD:] = 0.0
        Xc[SHARD:, 0] = 100.0  # fake far-away rows
        ss = np.ascontiguousarray((Xc * Xc).sum(axis=1, dtype=np.float32)[None, :])
        XcT = np.ascontiguousarray(Xc.T)
        hiT = XcT.astype(ml_dtypes.bfloat16)
        loT = (XcT - hiT.astype(np.float32)).astype(ml_dtypes.bfloat16)
        cofc = np.full((128, 1), float(c * SHARD), np.float32)
        in_maps.append({
            "xhiT": hiT, "xloT": loT, "ss_in": ss, "xq_in": x,
            "ltab": ltab, "p76_in": p76, "center": centr,
            "ident": ident, "dmask": dmask, "iota10": iota10,
            "coff": cofc,
        })

    nc = _get_nc()
    trace = os.environ.get("KTRACE") == "1"
    res = run_bass_kernel_spmd(nc, in_maps, list(range(NCORES)), trace=trace)
    global LAST_EXEC_NS
    LAST_EXEC_NS = res.exec_time_ns
    out = np.concatenate([res.results[c]["creds"] for c in range(NCORES)], axis=0)
    return out.astype(np.float32)


# revision 8
# speedup vs baseline: 1.0727x; 1.0727x over previous
"""DkNN retrieval kernel for 8 trn2 NeuronCores (self-contained).

Algorithm (matches reference.py):
  xq = x/||x|| - center;  score_j = ||X_j||^2 - 2 xq.X_j;  closest = argmin_j
  neigh = [closest, tni[closest]];  counts = bincount(labels[neigh]);
  p = (1000 - bisect_left(cali, 75-counts))/1000;  creds = onehot(argmax p)*max p

Distribution: X sharded over 8 cores on the train axis (12500 rows each,
padded to 12800 with far-away fake rows). Queries replicated. Matmuls use a
3-term bf16 split (hi*Hi + hi*Lo + lo*Hi) for ~2e-7 score accuracy (bf16
alone flips ~10 argmins; fp32r ~4.5e-5 error; fp32 native is 4x slower).

Host precomputes: row norms SS (replaces a 13MB fp32 X read + 100 Square
ops), the fused label table LTAB[j] = labels[[j, tni[j]]] (replaces the
neighbor-row gather + 75 per-slot label gathers with ONE indirect DMA), and
the conformal LUT p76[v] = (1000 - bisect_left(cali, v))/1000.

Device: X hi/lo preloaded to SBUF in 4 full-width DMAs; per (super, qtile)
6 bf16 matmuls accumulate -2*xq.X into a PSUM bank, then two custom DVE ops
read the bank directly: MINRED (body=ps+ss, accum MIN -> per-super min
value) and IDX_SCAN (reversed scan -> first argmin position). Cross-core
combine via AllToAll of (val, pos); tail (label counts + conformal
p-values) on the query-owning core.

HW quirks honored: indirect_copy gathers only from low SBUF addresses
(~<32KB absolute) -> p76 LUT tile allocated side="left"; indirect DMA
supports one offset per partition per call.
"""
import os
import numpy as np

import concourse.bass as bass
import concourse.bacc as bacc
import concourse.tile as tile
import concourse.mybir as mybir
import concourse.dve_ops as dve_ops_mod
from concourse.bass_utils import run_bass_kernel_spmd
from concourse.dve_ops import DveOp, OPS
from concourse.dve_spec import Spec, Src0, Src1, C0, MaxNeg, scan, select, eq, Idx, lower
from concourse.dve_uop import DveOpSpec, AluOp
from concourse.dve_table_gen import dve_ver_for

NB_DATA = 1024
NB_TRAIN = 100000
D = 256
NB_CALI = 1000
NCORES = 8

SHARD = 12500          # real candidates per core
SHARD_PAD = 12800      # padded (fake rows score ~+1e4, never win)
SUPER = 512            # candidate columns per PSUM super-tile (1 bank)
NSUP = 25              # 25*512 = 12800
QT = 8                 # query tiles of 128

_AluOp = mybir.AluOpType


def _register_dve(name, spec):
    if name in dve_ops_mod._SUB_OPCODE_FOR_NAME:
        for op in OPS:
            if op.name == name:
                return op
    opcode = dve_ops_mod._CUSTOM_DVE_ROW_BASE + len(OPS)
    dve_ops_mod._SUB_OPCODE_FOR_NAME[name] = opcode
    ver = dve_ver_for("TRN2")
    tmp = DveOpSpec(name=name, opcode=opcode, uops=lower(spec, ver=ver), rd1_en=True)
    op = DveOp(name, spec, subdim=False, uops_sha={ver: tmp.sha(ver)})
    OPS.append(op)
    return op


def _idx_scan_spec():
    s = Src0 + Src1
    r = scan(AluOp.MIN, s, init=C0)
    body = select(eq(s, r), Idx, MaxNeg)

    def ref(in0, in1, s0, s1, imm2):
        v = (in0.astype(np.float64) + in1.astype(np.float64)).astype(np.float32)
        rm = np.minimum(np.minimum.accumulate(v, axis=-1), np.float32(s0))
        idx = np.arange(v.shape[-1], dtype=np.float64)
        sel = np.where(v == rm, idx, -3.4e38)
        return sel.astype(np.float32)

    return Spec(body=body, accum=AluOp.MAX, reference=ref)


def _minred_spec():
    def ref(in0, in1, s0, s1, imm2):
        v = (in0.astype(np.float32) + in1.astype(np.float32))
        out = v.astype(np.float32)
        acc = np.minimum(np.min(v, axis=-1), np.float32(s0))
        return out, acc

    return Spec(body=Src0 + Src1, accum=AluOp.MIN, accum_init=C0, reference=ref)


IDX_SCAN = _register_dve("IDX_SCAN_ANT", _idx_scan_spec())
MINRED = _register_dve("MINRED_ANT", _minred_spec())
dt = mybir.dt


def build_kernel():
    PHASE = int(os.environ.get("KPHASE", "3"))
    nc = bacc.Bacc("TRN2", target_bir_lowering=False, debug=False,
                   num_devices=NCORES)

    # ---- I/O ----
    xhiT = nc.dram_tensor("xhiT", [D, SHARD_PAD], dt.bfloat16, kind="ExternalInput").ap()
    xloT = nc.dram_tensor("xloT", [D, SHARD_PAD], dt.bfloat16, kind="ExternalInput").ap()
    ss_in = nc.dram_tensor("ss_in", [1, SHARD_PAD], dt.float32, kind="ExternalInput").ap()
    xq_in = nc.dram_tensor("xq_in", [NB_DATA, D], dt.float32, kind="ExternalInput").ap()
    ltab = nc.dram_tensor("ltab", [NB_TRAIN, 75], dt.int32, kind="ExternalInput").ap()
    p76_in = nc.dram_tensor("p76_in", [1, 76], dt.float32, kind="ExternalInput").ap()
    center = nc.dram_tensor("center", [1, D], dt.float32, kind="ExternalInput").ap()
    ident = nc.dram_tensor("ident", [128, 128], dt.float32, kind="ExternalInput").ap()
    dmask = nc.dram_tensor("dmask", [128, 16], dt.float32, kind="ExternalInput").ap()
    iota10 = nc.dram_tensor("iota10", [128, 10], dt.float32, kind="ExternalInput").ap()
    coff = nc.dram_tensor("coff", [128, 1], dt.float32, kind="ExternalInput").ap()
    creds_out = nc.dram_tensor("creds", [128, 10], dt.float32, kind="ExternalOutput").ap()

    with tile.TileContext(nc) as tc:
        with tc.tile_pool(name="dram", bufs=1, space="DRAM") as dpool:
            loc_d = dpool.tile([NB_DATA, 2], dt.float32)
            glob_d = dpool.tile([NCORES, 128, 2], dt.float32)

            with tc.tile_pool(name="mp", bufs=1, side="right") as mp, \
                 tc.tile_pool(name="mp2", bufs=2, side="right") as mp2, \
                 tc.tile_pool(name="pp", bufs=1, space="PSUM") as pp:

                # ===== preload X hi/lo (4 max-width DMAs) + SS broadcast =====
                xh = [mp.tile([128, SHARD_PAD], dt.bfloat16, name=f"xh{k}") for k in range(2)]
                xl = [mp.tile([128, SHARD_PAD], dt.bfloat16, name=f"xl{k}") for k in range(2)]
                for k in range(2):
                    nc.sync.dma_start(xh[k][:], xhiT[k * 128:(k + 1) * 128, :])
                    nc.sync.dma_start(xl[k][:], xloT[k * 128:(k + 1) * 128, :])
                ssB = mp.tile([128, SHARD_PAD], dt.float32)
                nc.sync.dma_start(ssB[:], ss_in[0:1, :].to_broadcast([128, SHARD_PAD]))

                # ===== query prep =====
                cb = mp.tile([128, D], dt.float32)
                crow = mp.tile([1, D], dt.float32)
                nc.sync.dma_start(crow[:], center[:, :])
                nc.gpsimd.partition_broadcast(cb[:], crow[:])
                cb2 = mp.tile([128, D], dt.float32)
                nc.scalar.mul(out=cb2[:], in_=cb[:], mul=2.0)
                idt = mp.tile([128, 128], dt.float32)
                nc.sync.dma_start(idt[:], ident[:, :])

                xqTh = [mp.tile([128, NB_DATA], dt.bfloat16, name=f"xqTh{k}") for k in range(2)]
                xqTl = [mp.tile([128, NB_DATA], dt.bfloat16, name=f"xqTl{k}") for k in range(2)]
                for t in range(QT):
                    xt = mp2.tile([128, D], dt.float32, tag="xt", name=f"xt{t}")
                    nc.sync.dma_start(xt[:], xq_in[t * 128:(t + 1) * 128, :])
                    junk = mp2.tile([128, D], dt.float32, tag="junk", name=f"junk{t}")
                    ssq = mp2.tile([128, 1], dt.float32, tag="ssq", name=f"ssq{t}")
                    nc.scalar.activation(out=junk[:], in_=xt[:],
                                         func=mybir.ActivationFunctionType.Square,
                                         accum_out=ssq[:])
                    nrm = mp2.tile([128, 1], dt.float32, tag="nrm", name=f"nrm{t}")
                    nc.scalar.sqrt(out=nrm[:], in_=ssq[:])
                    rn = mp2.tile([128, 1], dt.float32, tag="rn", name=f"rn{t}")
                    nc.vector.reciprocal(out=rn[:], in_=nrm[:])
                    nc.vector.tensor_scalar(out=rn[:], in0=rn[:], scalar1=-2.0,
                                            scalar2=None, op0=_AluOp.mult)
                    xqp = mp2.tile([128, D], dt.float32, tag="xqp", name=f"xqp{t}")
                    nc.vector.scalar_tensor_tensor(
                        out=xqp[:], in0=xt[:], scalar=rn[:, 0:1], in1=cb2[:],
                        op0=_AluOp.mult, op1=_AluOp.add)
                    for k in range(2):
                        tpt = pp.tile([128, 4 * SUPER], dt.float32, tag="ps", bufs=2,
                                      name=f"tp{t}_{k}")
                        tp = tpt[:, 0:128]
                        nc.tensor.transpose(out=tp, in_=xqp[:, k * 128:(k + 1) * 128],
                                            identity=idt[:])
                        xqf = mp2.tile([128, 128], dt.float32, tag="xqf", name=f"xqf{t}_{k}")
                        nc.scalar.copy(out=xqf[:], in_=tp)
                        nc.vector.tensor_copy(out=xqTh[k][:, t * 128:(t + 1) * 128], in_=xqf[:])
                        nc.vector.tensor_tensor(
                            out=xqTl[k][:, t * 128:(t + 1) * 128],
                            in0=xqf[:], in1=xqTh[k][:, t * 128:(t + 1) * 128],
                            op=_AluOp.subtract)

                # ===== main loop: query-tile outer, 6 uniform groups of 4 =====
                # supers 0-23 in groups of 4 (2048 cols, 4 PSUM banks, bufs=2);
                # super 24 in a separate fine-grained tail pass below.
                NG = 7                     # 6 group cols + 1 tail col per tile
                VAL = mp.tile([128, QT * NG], dt.float32)
                POSG = mp.tile([128, QT * NG], dt.float32)
                terms = [(xqTh, xh), (xqTh, xl), (xqTl, xh)]

                def score_block(t, g, c0, w, name):
                    """matmuls + min/argmin DVE for cands [c0, c0+w), col t*NG+g."""
                    ps = pp.tile([128, 4 * SUPER], dt.float32, tag="ps", bufs=2,
                                 name=f"ps{name}")
                    # term-major: consecutive matmuls share one lhsT, so the
                    # repeats skip the 128-cycle weight load (ldweights=False)
                    nmm_last = len(terms) - 1
                    for nmm, (lhs, rhs) in enumerate(terms):
                        for k in range(2):
                            for j in range(w // SUPER):
                                mm = nc.tensor.matmul(
                                    ps[:, j * SUPER:(j + 1) * SUPER],
                                    lhs[k][:, t * 128:(t + 1) * 128],
                                    rhs[k][:, c0 + j * SUPER:c0 + (j + 1) * SUPER],
                                    start=(nmm == 0 and k == 0),
                                    stop=(nmm == nmm_last and k == 1))
                                if j > 0:
                                    mm.ins.ldweights = False
                    col = t * NG + g
                    mrd = mp2.tile([128, 4 * SUPER], dt.bfloat16, tag="mrd",
                                   name=f"mrd{name}")
                    nc.vector._custom_dve(
                        MINRED,
                        out=mrd[:, :w],
                        in0=ps[:, :w],
                        in1=ssB[:, c0:c0 + w],
                        s0=3.4e38,
                        accum_out=VAL[:, col:col + 1])
                    scr = mp2.tile([128, 4 * SUPER], dt.uint16, tag="scr",
                                   name=f"scr{name}")
                    posr = mp2.tile([128, 1], dt.float32, tag="posr",
                                    name=f"posr{name}")
                    nc.vector._custom_dve(
                        IDX_SCAN,
                        out=scr[:, :w][:, ::-1],
                        in0=ps[:, :w][:, ::-1],
                        in1=ssB[:, c0:c0 + w][:, ::-1],
                        s0=3.4e38,
                        accum_out=posr[:])
                    # true pos = (w-1) - reversed pos; global += c0
                    nc.vector.tensor_scalar(out=POSG[:, col:col + 1],
                                            in0=posr[:], scalar1=-1.0,
                                            scalar2=float(w - 1 + c0),
                                            op0=_AluOp.mult, op1=_AluOp.add)

                for t in range(QT):
                    for g in range(6):
                        score_block(t, g, g * 4 * SUPER, 4 * SUPER, f"{t}_{g}")
                # tail pass: super 24 for all tiles (fine-grained, pipelines)
                for t in range(QT):
                    score_block(t, 6, 24 * SUPER, SUPER, f"{t}_z")

                # ===== cross-group combine (per query-tile) =====
                gmin = mp.tile([128, 8], dt.float32)
                vview = VAL[:].rearrange("p (q s) -> p q s", q=8)
                nc.vector.tensor_reduce(gmin[:], vview, mybir.AxisListType.X,
                                        _AluOp.min)
                eqv = mp.tile([128, QT * NG], dt.uint8)
                nc.vector.tensor_tensor(
                    out=eqv[:].rearrange("p (q s) -> p q s", q=8),
                    in0=vview,
                    in1=gmin[:].unsqueeze(2).to_broadcast([128, 8, NG]),
                    op=_AluOp.is_equal)
                big = mp.tile([128, QT * NG], dt.float32)
                nc.gpsimd.memset(big[:], 1.0e9)
                selp = mp.tile([128, QT * NG], dt.float32)
                nc.vector.select(out=selp[:], mask=eqv[:], on_true=POSG[:],
                                 on_false=big[:])
                gpos = mp.tile([128, 8], dt.float32)
                nc.vector.tensor_reduce(gpos[:],
                                        selp[:].rearrange("p (q s) -> p q s", q=8),
                                        mybir.AxisListType.X, _AluOp.min)
                cof = mp.tile([128, 1], dt.float32)
                nc.sync.dma_start(cof[:], coff[:, :])
                nc.vector.tensor_scalar(out=gpos[:], in0=gpos[:],
                                        scalar1=cof[:, 0:1], scalar2=None,
                                        op0=_AluOp.add)
                locb = mp.tile([128, 16], dt.float32)
                nc.vector.tensor_copy(out=locb[:, 0::2], in_=gmin[:])
                nc.vector.tensor_copy(out=locb[:, 1::2], in_=gpos[:])
                for t in range(QT):
                    nc.sync.dma_start(loc_d[t * 128:(t + 1) * 128, :],
                                      locb[:, t * 2:t * 2 + 2])
                if PHASE == 1:
                    nc.sync.dma_start(creds_out[:, :], locb[:, :10])

            # ===== cross-core exchange + tail =====
            with tc.tile_pool(name="lo2", bufs=1, side="left") as lo2, \
                 tc.tile_pool(name="tp2", bufs=1, side="right") as tp2:
              if PHASE >= 2:
                nc.gpsimd.collective_compute(
                    "AllToAll",
                    _AluOp.bypass,
                    replica_groups=[list(range(NCORES))],
                    ins=[loc_d.opt()],
                    outs=[glob_d.opt()],
                )
                vi = tp2.tile([128, 16], dt.float32)
                nc.sync.dma_start(vi[:], glob_d[:].rearrange("r p e -> p r e"))
                vals8 = vi[:, 0::2]
                idx8 = vi[:, 1::2]
                m8 = tp2.tile([128, 1], dt.float32)
                nc.vector.tensor_reduce(m8[:], vals8, mybir.AxisListType.X,
                                        _AluOp.min)
                eq8 = tp2.tile([128, 8], dt.uint8)
                nc.vector.tensor_scalar(out=eq8[:], in0=vals8,
                                        scalar1=m8[:, 0:1], scalar2=None,
                                        op0=_AluOp.is_equal)
                big8 = tp2.tile([128, 8], dt.float32)
                nc.gpsimd.memset(big8[:], 1.0e9)
                sel8 = tp2.tile([128, 8], dt.float32)
                nc.vector.select(out=sel8[:], mask=eq8[:], on_true=idx8,
                                 on_false=big8[:])
                closf = tp2.tile([128, 1], dt.float32)
                nc.vector.tensor_reduce(closf[:], sel8[:], mybir.AxisListType.X,
                                        _AluOp.min)

                if PHASE >= 3:
                    closi = tp2.tile([128, 1], dt.int32)
                    nc.vector.tensor_copy(out=closi[:], in_=closf[:])
                    # labels of [closest, tni[closest]]: ONE row gather
                    labi = tp2.tile([128, 75], dt.int32)
                    nc.gpsimd.indirect_dma_start(
                        out=labi[:, :], out_offset=None, in_=ltab[:, :],
                        in_offset=bass.IndirectOffsetOnAxis(ap=closi[:, 0:1], axis=0))
                    labs = tp2.tile([128, 75], dt.float32)
                    nc.vector.tensor_copy(out=labs[:], in_=labi[:])

                    counts = tp2.tile([128, 10], dt.float32)
                    junk75 = tp2.tile([128, 75], dt.float32)
                    for c in range(10):
                        nc.vector.scalar_tensor_tensor(
                            out=junk75[:], in0=labs[:], scalar=float(c),
                            in1=labs[:], op0=_AluOp.is_equal, op1=_AluOp.bypass,
                            accum_out=counts[:, c:c + 1])
                    knn = tp2.tile([128, 10], dt.float32)
                    nc.vector.tensor_scalar(out=knn[:], in0=counts[:], scalar1=-1.0,
                                            scalar2=75.0, op0=_AluOp.mult,
                                            op1=_AluOp.add)

                    # conformal LUT (host-computed): p76[v] = (1000 - #(cali<v))/1000
                    p76r = tp2.tile([1, 76], dt.float32)
                    nc.sync.dma_start(p76r[:], p76_in[:, :])
                    p76b = lo2.tile([128, 76], dt.float32)  # low SBUF for gather
                    nc.gpsimd.partition_broadcast(p76b[:], p76r[:])

                    knn16 = tp2.tile([128, 10], dt.uint16)
                    nc.vector.tensor_copy(out=knn16[:], in_=knn[:])
                    gp = tp2.tile([128, 160], dt.float32)
                    nc.gpsimd.indirect_copy(out=gp[:], data=p76b[:], idxs=knn16[:],
                                            i_know_ap_gather_is_preferred=True)
                    dmt2 = tp2.tile([128, 16], dt.float32)
                    nc.sync.dma_start(dmt2[:], dmask[:, :])
                    nc.vector.tensor_tensor(
                        out=gp[:].rearrange("p (a b) -> p a b", b=16),
                        in0=gp[:].rearrange("p (a b) -> p a b", b=16),
                        in1=dmt2[:].unsqueeze(1).to_broadcast([128, 10, 16]),
                        op=_AluOp.mult)
                    pval = tp2.tile([128, 10], dt.float32)
                    nc.vector.tensor_reduce(pval[:],
                                            gp[:].rearrange("p (a b) -> p a b", b=16),
                                            mybir.AxisListType.X, _AluOp.add)

                    m10 = tp2.tile([128, 1], dt.float32)
                    nc.vector.tensor_reduce(m10[:], pval[:], mybir.AxisListType.X,
                                            _AluOp.max)
                    eqp = tp2.tile([128, 10], dt.uint8)
                    nc.vector.tensor_scalar(out=eqp[:], in0=pval[:],
                                            scalar1=m10[:, 0:1], scalar2=None,
                                            op0=_AluOp.is_equal)
                    io10 = tp2.tile([128, 10], dt.float32)
                    nc.sync.dma_start(io10[:], iota10[:, :])
                    big10 = tp2.tile([128, 10], dt.float32)
                    nc.gpsimd.memset(big10[:], 1.0e9)
                    candp = tp2.tile([128, 10], dt.float32)
                    nc.vector.select(out=candp[:], mask=eqp[:], on_true=io10[:],
                                     on_false=big10[:])
                    pred = tp2.tile([128, 1], dt.float32)
                    nc.vector.tensor_reduce(pred[:], candp[:], mybir.AxisListType.X,
                                            _AluOp.min)
                    cmask = tp2.tile([128, 10], dt.uint8)
                    nc.vector.tensor_scalar(out=cmask[:], in0=io10[:],
                                            scalar1=pred[:, 0:1], scalar2=None,
                                            op0=_AluOp.is_equal)
                    cmf = tp2.tile([128, 10], dt.float32)
                    nc.vector.tensor_copy(out=cmf[:], in_=cmask[:])
                    credst = tp2.tile([128, 10], dt.float32)
                    nc.vector.tensor_scalar(out=credst[:], in0=cmf[:],
                                            scalar1=m10[:, 0:1], scalar2=None,
                                            op0=_AluOp.mult)
                    nc.sync.dma_start(creds_out[:, :], credst[:])
                if PHASE == 2:
                    credst = tp2.tile([128, 10], dt.float32, name="credst2")
                    nc.gpsimd.memset(credst[:], 0.0)
                    nc.vector.tensor_copy(out=credst[:, 0:1], in_=closf[:])
                    nc.vector.tensor_copy(out=credst[:, 1:2], in_=m8[:])
                    nc.sync.dma_start(creds_out[:, :], credst[:])

    nc.compile()
    return nc


_NC_CACHE = None
LAST_EXEC_NS = None


def _get_nc():
    global _NC_CACHE
    if _NC_CACHE is None:
        _NC_CACHE = build_kernel()
    return _NC_CACHE


def kernel(x, X, center, train_labels, train_neighbor_index, cali_nonconformity):
    x = np.ascontiguousarray(np.asarray(x, dtype=np.float32))
    X = np.ascontiguousarray(np.asarray(X, dtype=np.float32))
    center = np.asarray(center, dtype=np.float32)
    tni = np.ascontiguousarray(np.asarray(train_neighbor_index, dtype=np.int32))
    labels = np.asarray(train_labels, dtype=np.int32)
    cali = np.asarray(cali_nonconformity, dtype=np.int32)

    import ml_dtypes

    dmask = np.zeros((128, 16), np.float32)
    for p in range(128):
        dmask[p, p % 16] = 1.0
    iota10 = np.broadcast_to(np.arange(10, dtype=np.float32), (128, 10)).copy()
    ident = np.eye(128, dtype=np.float32)
    calif = cali.astype(np.float32)
    centr = np.ascontiguousarray(center[None, :])

    # labels of [j, tni[j]] fused into one gatherable table
    ltab = np.ascontiguousarray(
        labels[np.concatenate([np.arange(NB_TRAIN, dtype=np.int32)[:, None], tni],
                              axis=1)])
    # conformal LUT over the 76 possible nonconformity values
    pos76 = np.searchsorted(cali, np.arange(76, dtype=np.int32), side='left')
    p76 = np.ascontiguousarray(
        ((NB_CALI - pos76).astype(np.float32) / float(NB_CALI))[None, :])

    in_maps = []
    for c in range(NCORES):
        Xc = np.empty((SHARD_PAD, D), np.float32)
        Xc[:SHARD] = X[c * SHARD:(c + 1) * SHARD]
        Xc[SHARD:] = 0.0
        Xc[SHARD:, 0] = 100.0  # fake far-away rows
        ss = np.ascontiguousarray((Xc * Xc).sum(axis=1, dtype=np.float32)[None, :])
        XcT = np.ascontiguousarray(Xc.T)
        hiT = XcT.astype(ml_dtypes.bfloat16)
        loT = (XcT - hiT.astype(np.float32)).astype(ml_dtypes.bfloat16)
        cofc = np.full((128, 1), float(c * SHARD), np.float32)
        in_maps.append({
            "xhiT": hiT, "xloT": loT, "ss_in": ss, "xq_in": x,
            "ltab": ltab, "p76_in": p76, "center": centr,
            "ident": ident, "dmask": dmask, "iota10": iota10,
            "coff": cofc,
        })

    nc = _get_nc()
    trace = os.environ.get("KTRACE") == "1"
    res = run_bass_kernel_spmd(nc, in_maps, list(range(NCORES)), trace=trace)
    global LAST_EXEC_NS
    LAST_EXEC_NS = res.exec_time_ns
    out = np.concatenate([res.results[c]["creds"] for c in range(NCORES)], axis=0)
    return out.astype(np.float32)
